# revision 1
# baseline (speedup 1.0000x reference)
"""Causal multi-head attention on 8 trn2 NeuronCores.

Problem: B=2, S=2048, D=1024, H=16 heads, HD=64. fp32 in/out.

Sharding: 8 cores = 2 (batch) x 4 (head groups of 4 heads).
Each core computes, for its batch b and head group g:
  Q^T,K^T  [256, 2048]  (d on partitions, seq on free)  = W^T-slice x
  V        [2048, 256+ones]  (natural, with a ones column per head)
  per 512-wide q chunk, per head:  S^T[k,q] = K^T.T @ Q^T  (PE, contraction 64,
  2-head row-packed), P~ = exp(S^T/8) (ACT), causal via block skipping +
  one gpsimd affine_select per diagonal block, PV: out^T[d,q] accumulated
  over k tiles with V_aug stationary (m=65; row 64 = softmax denominator).
  Divide by denominator (DVE reciprocal + mul, gpsimd partition_broadcast),
  then O_partial = ctx^T.T @ Wo_rows  [2048, 1024].
Host: sums the 4 head-group partials per batch and adds bo + bv @ Wo
(exact: the bv bias contributes the constant row vector bv @ Wo_g).

Default mode "f16in": x/Wq/Wk/Wv ship as fp16 (halves the dominant DMA
traffic; QKV still accumulates in fp32 PSUM), everything downstream uses
float32r matmuls (full 1-cycle/row PE rate at N>=256, ~14-bit mantissa).

Emission schedule: DMAs ordered so chunk-0 dependencies land first; per
q-chunk [V proj, QK proj, attention] interleaved so ACT exp overlaps the
next chunk's PE projections; all Wo projections emitted last (they fill
PE gaps; keeping them out of the per-chunk stream avoids psum pool-slot
blocking of later projections).

Measured on HW: rel err 5.1e-4 vs fp32 reference; ~264us/iteration
single-core, ~270-300us with all 8 cores active (HBM contention).
"""

import sys

if "/opt/trn_rl_repo" not in sys.path:
    sys.path.insert(0, "/opt/trn_rl_repo")

import numpy as np

import concourse.bacc as bacc
import concourse.bass as bass
import concourse.mybir as mybir
import concourse.tile as tile
from concourse.bass_utils import run_bass_kernel_spmd

B, S, D, H = 2, 2048, 1024, 16
HD = D // H  # 64
N_CORES = 8
HEADS_PER_CORE = H // 4  # 4
DG = HEADS_PER_CORE * HD  # 256 head dims per core
P = 128
CHUNK = 512  # q chunk width
N_KT = S // P  # 16 k tiles
N_CH = S // CHUNK  # 4 q chunks
F32 = mybir.dt.float32

_CACHE = {}


def _mm(dt_name):
    return {"f32r": mybir.dt.float32r, "f16in": mybir.dt.float32r,
            "f32": mybir.dt.float32,
            "bf16": mybir.dt.bfloat16}[dt_name]


def _in_dt(dt_name):
    """dtype for the x / Wq / Wk / Wv inputs (DMA-traffic dominant)."""
    return mybir.dt.float16 if dt_name == "f16in" else _mm(dt_name)


def build_kernel(mm_dt="f32r", unroll=1, ablate=()):
    """Build + compile the per-core SPMD program. unroll>1 wraps the body
    in a hardware loop (for pure device timing measurements)."""
    mdt = _mm(mm_dt)
    idt = _in_dt(mm_dt)

    nc = bacc.Bacc("TRN2", target_bir_lowering=False, debug=False)
    xT_d = nc.dram_tensor("xT", [D, S], idt, kind="ExternalInput")
    wq_d = nc.dram_tensor("wq", [D, DG], idt, kind="ExternalInput")
    wk_d = nc.dram_tensor("wk", [D, DG], idt, kind="ExternalInput")
    wv_d = nc.dram_tensor("wv", [D, DG], idt, kind="ExternalInput")
    wo_d = nc.dram_tensor("wo", [DG, D], mdt, kind="ExternalInput")
    bq_d = nc.dram_tensor("bq", [DG, 1], F32, kind="ExternalInput")
    bk_d = nc.dram_tensor("bk", [DG, 1], F32, kind="ExternalInput")
    o_d = nc.dram_tensor("o", [S, D], F32, kind="ExternalOutput")

    NDT = D // P  # 8 contraction tiles over D
    NMT = DG // P  # 2 m-tiles over the core's head dims (= head pairs)

    with tile.TileContext(nc) as tc:
        def body(_iv=None):
            _body(tc, nc, mdt, idt,
                  xT_d, wq_d, wk_d, wv_d, wo_d, bq_d, bk_d, o_d, NDT, NMT,
                  ablate)

        if unroll > 1:
            with tc.For_i(0, unroll, 1):
                body()
        else:
            body()

    nc.compile()
    return nc


def _body(tc, nc, mdt, idt, xT_d, wq_d, wk_d, wv_d, wo_d, bq_d, bk_d, o_d,
          NDT, NMT, ablate=()):
    import contextlib
    ctx = contextlib.ExitStack()
    with ctx:
        const = ctx.enter_context(tc.tile_pool(name="const", bufs=1))
        sbuf = ctx.enter_context(tc.tile_pool(name="sbuf", bufs=1))
        ptile_p = ctx.enter_context(tc.tile_pool(name="ptile", bufs=8))
        den_p = ctx.enter_context(tc.tile_pool(name="den", bufs=3))
        out_p = ctx.enter_context(tc.tile_pool(name="outp", bufs=3))
        qkv_ps = ctx.enter_context(
            tc.tile_pool(name="qkv_ps", bufs=2, space="PSUM"))
        stp_ps = ctx.enter_context(
            tc.tile_pool(name="stp_ps", bufs=2, space="PSUM"))
        pv_ps = ctx.enter_context(
            tc.tile_pool(name="pv_ps", bufs=2, space="PSUM"))

        # ---- load inputs ------------------------------------------------
        # weights/biases first, then xt in chunk-major slices so chunk-0
        # compute starts early; all loads are queued before any output
        # stores (the HWDGE queue is in-order)
        xt = []
        for i in range(NDT):
            t = const.tile([P, S], idt, tag=f"xt{i}", name=f"xt{i}")
            xt.append(t)
        ws = {}
        for name, d in (("wq", wq_d), ("wk", wk_d), ("wv", wv_d)):
            ws[name] = [const.tile([P, DG], idt, tag=f"{name}{i}",
                                   name=f"{name}{i}") for i in range(NDT)]
        wo = [const.tile([P, D], mdt, tag=f"wo{m}", name=f"wo{m}")
              for m in range(NMT)]
        biases = {(name, m): const.tile([P, 1], F32, tag=f"{name}{m}",
                                        name=f"{name}{m}")
                  for name in ("bq", "bk") for m in range(NMT)}

        def dma_w(name, d):
            for i in range(NDT):
                nc.sync.dma_start(ws[name][i][:],
                                  d.ap()[P * i:P * (i + 1), :])

        def dma_xt(ci):
            csl = slice(CHUNK * ci, CHUNK * (ci + 1))
            for k in range(NDT):
                nc.sync.dma_start(xt[k][:, csl],
                                  xT_d.ap()[P * k:P * (k + 1), csl])

        # order: V(0)+QK(0) deps first, then remaining chunks, wo last
        dma_w("wv", wv_d)
        dma_xt(0)
        dma_w("wq", wq_d)
        dma_w("wk", wk_d)
        for (name, m), t in biases.items():
            d = bq_d if name == "bq" else bk_d
            nc.sync.dma_start(t[:], d.ap()[P * m:P * (m + 1), :])
        for ci in range(1, N_CH):
            dma_xt(ci)
        for m in range(NMT):
            nc.sync.dma_start(wo[m][:], wo_d.ap()[P * m:P * (m + 1), :])

        # ---- V projection (natural layout + ones cols) ------------------
        # vaug[j]: [128, 4*65]; per head h cols h*65..h*65+63 = V, col h*65+64 = 1
        ones_f = const.tile([P, HEADS_PER_CORE], F32, tag="ones_f",
                            name="ones_f")
        nc.vector.memset(ones_f[:], 1.0)
        ones_r = const.tile([P, HEADS_PER_CORE], mdt, tag="ones_r",
                            name="ones_r")
        nc.vector.tensor_copy(ones_r[:], ones_f[:])
        vaug = []
        for j in range(N_KT):
            t = sbuf.tile([P, HEADS_PER_CORE * (HD + 1)], mdt, tag=f"vaug{j}", name=f"vaug{j}")
            vaug.append(t)

        def v_proj(j):
            ps = qkv_ps.tile([P, CHUNK], F32, tag="proj", name="proj")
            for k in range(NDT):
                nc.tensor.matmul(
                    ps[:, 0:DG],
                    xt[k][:, P * j:P * (j + 1)],
                    ws["wv"][k][:],
                    start=(k == 0), stop=(k == NDT - 1))
            dst = vaug[j][:].rearrange("p (h x) -> p h x", h=HEADS_PER_CORE)
            srcp = ps[:, 0:DG].rearrange("p (h x) -> p h x", h=HEADS_PER_CORE)
            nc.vector.tensor_copy(dst[:, :, 0:HD], srcp[:, :, :])
            nc.vector.tensor_copy(
                dst[:, :, HD:HD + 1],
                ones_r[:].rearrange("p (h x) -> p h x", x=1))

        # ---- Q^T / K^T projections (d on partitions) --------------------
        qt, kt = [], []
        for name, lst in (("wq", qt), ("wk", kt)):
            for m in range(NMT):
                t = sbuf.tile([P, S], mdt, tag=f"{name}T{m}", name=f"{name}T{m}")
                lst.append(t)
        def qk_proj(ci):
            for name, lst in (("wq", qt), ("wk", kt)):
                bname = "bq" if name == "wq" else "bk"
                for m in range(NMT):
                    ps = qkv_ps.tile([P, CHUNK], F32, tag="proj", name="proj")
                    for k in range(NDT):
                        nc.tensor.matmul(
                            ps[:],
                            ws[name][k][:, P * m:P * (m + 1)],
                            xt[k][:, CHUNK * ci:CHUNK * (ci + 1)],
                            start=(k == 0), stop=(k == NDT - 1))
                    nc.vector.tensor_scalar_add(
                        lst[m][:, CHUNK * ci:CHUNK * (ci + 1)], ps[:],
                        biases[(bname, m)][:])

        # ---- attention + output projection, per q chunk -----------------
        ctxT = [sbuf.tile([P, S], mdt, tag=f"ctxT{m}", name=f"ctxT{m}") for m in range(NMT)]


        wo_work = []
        for ci in range(N_CH):
            for j in range(4 * ci, 4 * ci + 4):
                v_proj(j)
            qk_proj(ci)
            jmax = 4 * ci + 3  # last valid k tile for this chunk
            qsl = slice(CHUNK * ci, CHUNK * (ci + 1))
            for pair in range(NMT):
                pv = [pv_ps.tile([HD + 1, CHUNK], F32, tag="pv", name="pv")
                      for _ in range(2)]
                for j0 in range(0, jmax + 1, 2):
                    js = [j for j in (j0, j0 + 1) if j <= jmax]
                    nj = len(js)
                    pt = {}
                    for hh in range(2):  # head within pair
                        psl = slice(64 * hh, 64 * (hh + 1))
                        st = stp_ps.tile([P, 2 * CHUNK], F32, tag="stp",
                                         name="stp")
                        for gi, j in enumerate(js):
                            nc.tensor.matmul(
                                st[:, CHUNK * gi:CHUNK * (gi + 1)],
                                kt[pair][psl, P * j:P * (j + 1)],
                                qt[pair][psl, qsl],
                                start=True, stop=True)
                        p_t = ptile_p.tile([P, 2 * CHUNK], mdt, tag="ptile",
                                           name="ptile")
                        if "exp" in ablate:
                            nc.vector.tensor_copy(
                                p_t[:, 0:CHUNK * nj], st[:, 0:CHUNK * nj])
                        else:
                            nc.scalar.activation(
                                p_t[:, 0:CHUNK * nj], st[:, 0:CHUNK * nj],
                                mybir.ActivationFunctionType.Exp,
                                scale=0.125)
                        for gi, j in enumerate(js):
                            if j >= 4 * ci and "mask" not in ablate:
                                dd = j - 4 * ci
                                w = P * (dd + 1)
                                base = CHUNK * gi
                                nc.gpsimd.affine_select(
                                    out=p_t[:, base:base + w],
                                    in_=p_t[:, base:base + w],
                                    compare_op=mybir.AluOpType.is_ge,
                                    fill=0.0, base=-P * dd,
                                    pattern=[[1, w]],
                                    channel_multiplier=-1)
                        pt[hh] = p_t
                    for gi, j in enumerate(js):
                        for hh in range(2):
                            h = 2 * pair + hh
                            nc.tensor.matmul(
                                pv[hh][:],
                                vaug[j][:, (HD + 1) * h:(HD + 1) * (h + 1)],
                                pt[hh][:, CHUNK * gi:CHUNK * (gi + 1)],
                                start=(j == 0), stop=(j == jmax))
                # softmax denominator divide; write ctx^T chunk
                # (partition_broadcast only reaches partitions 0-63, so use
                # a base-0 tile per head; DVE ops allow mismatched bases)
                if "div" in ablate:
                    for hh in range(2):
                        nc.vector.tensor_copy(
                            ctxT[pair][64 * hh:64 * (hh + 1), qsl],
                            pv[hh][0:HD, :])
                else:
                    den_t = den_p.tile([1, 2 * CHUNK], F32, tag="den",
                                       name="den")
                    for hh in range(2):
                        nc.vector.tensor_copy(
                            den_t[0:1, CHUNK * hh:CHUNK * (hh + 1)],
                            pv[hh][HD:HD + 1, :])
                    nc.vector.reciprocal(den_t[:], den_t[:])
                    for hh in range(2):
                        recb = den_p.tile([HD, CHUNK], F32,
                                          tag=f"recb{hh}", name=f"recb{hh}")
                        nc.gpsimd.partition_broadcast(
                            recb[0:HD, :],
                            den_t[0:1, CHUNK * hh:CHUNK * (hh + 1)])
                        nc.vector.tensor_mul(
                            ctxT[pair][64 * hh:64 * (hh + 1), qsl],
                            pv[hh][0:HD, :],
                            recb[0:HD, :])
            wo_work.append(ci)

        # ---- Wo projections, emitted last (uses idle PE slots) ----------
        for ci in wo_work:
            for qi in range(4):
                i = 4 * ci + qi
                ot = out_p.tile([P, D], F32, tag="ot", name="ot")
                for e in range(2):
                    ps = qkv_ps.tile([P, CHUNK], F32, tag="proj", name="proj")
                    for m in range(NMT):
                        nc.tensor.matmul(
                            ps[:],
                            ctxT[m][:, P * i:P * (i + 1)],
                            wo[m][:, CHUNK * e:CHUNK * (e + 1)],
                            start=(m == 0), stop=(m == NMT - 1))
                    nc.any.tensor_copy(ot[:, CHUNK * e:CHUNK * (e + 1)],
                                       ps[:])
                nc.sync.dma_start(o_d.ap()[P * i:P * (i + 1), :], ot[:])


def _shard_inputs(x, Wq, bq, Wk, bk, Wv, bv, Wo, bo):
    mm_dt = _CACHE.get("mm_dt", "f16in")
    ndt = np.float16 if mm_dt == "f16in" else np.float32
    x = np.asarray(x, np.float32)
    in_maps = []
    for core in range(N_CORES):
        b, g = divmod(core, 4)
        ds = slice(DG * g, DG * (g + 1))
        in_maps.append({
            "xT": np.ascontiguousarray(x[b].T).astype(ndt),
            "wq": np.ascontiguousarray(
                np.asarray(Wq, np.float32)[:, ds]).astype(ndt),
            "wk": np.ascontiguousarray(
                np.asarray(Wk, np.float32)[:, ds]).astype(ndt),
            "wv": np.ascontiguousarray(
                np.asarray(Wv, np.float32)[:, ds]).astype(ndt),
            "wo": np.ascontiguousarray(np.asarray(Wo, np.float32)[ds, :]),
            "bq": np.asarray(bq, np.float32)[ds].reshape(DG, 1).copy(),
            "bk": np.asarray(bk, np.float32)[ds].reshape(DG, 1).copy(),
        })
    return in_maps


def kernel(x, Wq, bq, Wk, bk, Wv, bv, Wo, bo):
    mm_dt = _CACHE.get("mm_dt", "f16in")
    _CACHE["mm_dt"] = mm_dt
    if "nc" not in _CACHE:
        _CACHE["nc"] = build_kernel(mm_dt)
    nc = _CACHE["nc"]
    in_maps = _shard_inputs(x, Wq, bq, Wk, bk, Wv, bv, Wo, bo)
    res = run_bass_kernel_spmd(
        nc, in_maps, core_ids=list(range(N_CORES)), trace=False)
    out = np.zeros((B, S, D), np.float32)
    for core in range(N_CORES):
        out[core // 4] += res.results[core]["o"]
    # exact bias folding: +bo, + bv @ Wo (constant row vector)
    out += (np.asarray(bo, np.float32)
            + np.asarray(bv, np.float32) @ np.asarray(Wo, np.float32))
    return out



# revision 29
# speedup vs baseline: 1.3520x; 1.3520x over previous
"""Causal multi-head attention on 8 trn2 NeuronCores.

Problem: B=2, S=2048, D=1024, H=16 heads, HD=64. fp32 in/out.

Sharding: 8 cores = 2 (batch) x 4 (head groups of 4 heads).

v2 design (per core, batch b / head group g):
  - Projections run in fp8e4m3 with MatmulPerfMode.DoubleRow (0.5 cyc/row):
    host ships x^T and Wq/Wk/Wv in the paired [64, 2, *] layout (contraction
    rows d=128t+2p+i on partition p slot i).  Weights are scaled by 32 so
    fp8 stays in its normal range; the 32*32 factor on Q.K is folded into
    the exp scale (2^-13 exact), and the 32 on V is cancelled by using 32.0
    as the ones-column of V_aug (so den = 32*sum(p)) -- all exact.
  - Attention operands (Q^T/K^T/P/V/ctx^T) are fp16 (1 cyc/row at any N).
  - Causal handling: per 512-wide q chunk, k tiles j<4ci are full; the 4
    diagonal j's are processed in pairs trimmed to the valid q suffix
    (width W=512-128*dd0, packed compactly so exp is one ACT call), with
    one gpsimd affine_select per diagonal j zeroing the remaining triangle.
  - Softmax denominator: V_aug ones column -> pv psum row 64; DVE copies the
    two rows of a head pair into a [2,512] tile, one batched reciprocal,
    gpsimd partition_broadcast to [64,512], DVE multiply writes ctx^T fp16.
  - Output: Wo matmuls accumulate in PSUM and DMA straight from PSUM to HBM
    (f32), no SBUF staging.
  - Emission schedule: software pipeline; PV(group) is emitted after
    ST(next group) with projection/Wo chains woven between as PE filler so
    the Tensor engine never waits on ACT exp.
Host: sums the 4 head-group partials per batch and adds bo + bv @ Wo.
"""

import sys

if "/opt/trn_rl_repo" not in sys.path:
    sys.path.insert(0, "/opt/trn_rl_repo")

import numpy as np

import concourse.bacc as bacc
import concourse.bass as bass
import concourse.mybir as mybir
import concourse.tile as tile
from concourse.bass_utils import run_bass_kernel_spmd

B, S, D, H = 2, 2048, 1024, 16
HD = D // H  # 64
N_CORES = 8
HEADS_PER_CORE = H // 4  # 4
DG = HEADS_PER_CORE * HD  # 256 head dims per core
P = 128
CHUNK = 512  # q chunk width
N_KT = S // P  # 16 k tiles
N_CH = S // CHUNK  # 4 q chunks
NDT = D // P  # 8 contraction tiles over D
NMT = DG // P  # 2 m-tiles (head pairs)
F32 = mybir.dt.float32
F16 = mybir.dt.float16
FP8 = mybir.dt.float8e4
WSCALE = 32.0  # fp8 weight scale (exactly compensated downstream)
EXP_SCALE = 0.125 / (WSCALE * WSCALE)  # 2^-13

_CACHE = {}


def build_kernel(mm_dt="f16", unroll=1, ablate=()):
    fp8 = mm_dt == "fp8dr"
    nc = bacc.Bacc("TRN2", target_bir_lowering=False, debug=False)
    if fp8:
        x_d = nc.dram_tensor("xd", [P // 2, 2 * S * NDT], FP8,
                             kind="ExternalInput")
        w_d = nc.dram_tensor("wcat", [P // 2, 2 * 3 * DG * NDT], FP8,
                             kind="ExternalInput")
    else:
        x_d = nc.dram_tensor("xd", [D, S], F16, kind="ExternalInput")
        w_d = nc.dram_tensor("wcat", [D, 3 * DG], F16, kind="ExternalInput")
    wo_d = nc.dram_tensor("wo", [DG, D], F16, kind="ExternalInput")
    b_d = nc.dram_tensor("bcat", [P, 2 * NMT], F32, kind="ExternalInput")
    o_d = nc.dram_tensor("o", [S, D], F16, kind="ExternalOutput")

    with tile.TileContext(nc) as tc:
        def body(_iv=None):
            _body(tc, nc, fp8, x_d, w_d, wo_d, b_d, o_d, ablate)

        if unroll > 1:
            with tc.For_i(0, unroll, 1):
                body()
        else:
            body()

    nc.compile()
    return nc


def _body(tc, nc, fp8, x_d, w_d, wo_d, b_d, o_d, ablate=()):
    import contextlib
    ctx = contextlib.ExitStack()
    DR = mybir.MatmulPerfMode.DoubleRow if fp8 else None
    idt = FP8 if fp8 else F16
    with ctx:
        const = ctx.enter_context(tc.tile_pool(name="const", bufs=1))
        sbuf = ctx.enter_context(tc.tile_pool(name="sbuf", bufs=1))
        ptile_p = ctx.enter_context(tc.tile_pool(
            name="ptile", bufs=_CACHE.get("lookahead", 3) + 2))
        den_p = ctx.enter_context(tc.tile_pool(name="den", bufs=2))
        out_p = ctx.enter_context(tc.tile_pool(name="outp", bufs=3))
        pv_bufs, qkv_bufs = _CACHE.get("psum_cfg", (2, 2))
        stp_ps = ctx.enter_context(
            tc.tile_pool(name="stp_ps", bufs=2, space="PSUM"))
        pv_ps = ctx.enter_context(
            tc.tile_pool(name="pv_ps", bufs=pv_bufs, space="PSUM"))
        qkv_ps = ctx.enter_context(
            tc.tile_pool(name="qkv_ps", bufs=qkv_bufs, space="PSUM"))

        # ---- input tiles -------------------------------------------------
        # wq/wk/wv ship concatenated ([.., 3*DG] per contraction tile) so
        # each k-tile is ONE dma; x tiles are one dma each.  Loads alternate
        # between the two HWDGE queues (SP + Activation) t-interleaved so
        # the t=0 tiles land first; stores go on SP.
        WOFF = {"wv": 0, "wq": DG, "wk": 2 * DG}
        if fp8:
            xt = [const.tile([P // 2, 2, S], FP8, tag=f"xt{t}", name=f"xt{t}")
                  for t in range(NDT)]
            wct = [const.tile([P // 2, 2, 3 * DG], FP8, tag=f"wc{t}",
                              name=f"wc{t}") for t in range(NDT)]
            ws = {name: [wct[t][:, :, WOFF[name]:WOFF[name] + DG]
                         for t in range(NDT)]
                  for name in ("wq", "wk", "wv")}
        else:
            xt = [const.tile([P, S], F16, tag=f"xt{t}", name=f"xt{t}")
                  for t in range(NDT)]
            wct = [const.tile([P, 3 * DG], F16, tag=f"wc{t}",
                              name=f"wc{t}") for t in range(NDT)]
            ws = {name: [wct[t][:, WOFF[name]:WOFF[name] + DG]
                         for t in range(NDT)]
                  for name in ("wq", "wk", "wv")}
        wo = [const.tile([P, D], F16, tag=f"wo{m}", name=f"wo{m}")
              for m in range(NMT)]
        bcat = const.tile([P, 2 * NMT], F32, tag="bcat", name="bcat")
        biases = {(nm, m): bcat[:, i:i + 1]
                  for i, (nm, m) in enumerate(
                      (n, m) for n in ("bq", "bk") for m in range(NMT))}

        for t in range(NDT):
            eng = nc.sync if t % 2 == 0 else nc.scalar
            if fp8:
                wsrc = w_d.ap().rearrange("p (t two g) -> p t two g",
                                          t=NDT, two=2)[:, t]
                xsrc = x_d.ap().rearrange("p (t two s) -> p t two s",
                                          t=NDT, two=2)[:, t]
            else:
                wsrc = w_d.ap()[P * t:P * (t + 1), :]
                xsrc = x_d.ap()[P * t:P * (t + 1), :]
            eng.dma_start(wct[t][:], wsrc)
            eng.dma_start(xt[t][:], xsrc)
        nc.scalar.dma_start(bcat[:], b_d.ap()[:])
        for m in range(NMT):
            nc.scalar.dma_start(wo[m][:], wo_d.ap()[P * m:P * (m + 1), :])

        # ---- persistent sbuf tensors ------------------------------------
        qt = [sbuf.tile([P, S], F16, tag=f"qT{m}", name=f"qT{m}")
              for m in range(NMT)]
        kt = [sbuf.tile([P, S], F16, tag=f"kT{m}", name=f"kT{m}")
              for m in range(NMT)]
        ctxT = [sbuf.tile([P, S], F16, tag=f"ctxT{m}", name=f"ctxT{m}")
                for m in range(NMT)]
        vaug = [sbuf.tile([P, HEADS_PER_CORE, HD + 1], F16, tag=f"vaug{j}",
                          name=f"vaug{j}") for j in range(N_KT)]
        ones16 = const.tile([P, HEADS_PER_CORE, 1], F16, tag="ones16",
                            name="ones16")
        nc.vector.memset(ones16[:], WSCALE)

        # ---- projection / output chains (PE filler units) ---------------
        def mm(ps, lhsT, rhs, start, stop):
            nc.tensor.matmul(ps, lhsT, rhs, start=start, stop=stop,
                             perf_mode=DR)

        def v_proj(j):
            ps = qkv_ps.tile([P, CHUNK], F32, tag="proj", name="proj")
            for t in range(NDT):
                if fp8:
                    lhsT = xt[t][:, :, P * j:P * (j + 1)]
                    rhs = ws["wv"][t][:]
                else:
                    lhsT = xt[t][:, P * j:P * (j + 1)]
                    rhs = ws["wv"][t][:]
                mm(ps[:, 0:DG], lhsT, rhs, t == 0, t == NDT - 1)
            dst = vaug[j][:]
            srcp = ps[:, 0:DG].rearrange("p (h x) -> p h x",
                                         h=HEADS_PER_CORE)
            nc.vector.tensor_copy(dst[:, :, 0:HD], srcp)
            nc.vector.tensor_copy(dst[:, :, HD:HD + 1], ones16[:])

        def qk_proj(name, m, ci):
            lst = qt if name == "wq" else kt
            bname = "bq" if name == "wq" else "bk"
            csl = slice(CHUNK * ci, CHUNK * (ci + 1))
            ps = qkv_ps.tile([P, CHUNK], F32, tag="proj", name="proj")
            for t in range(NDT):
                if fp8:
                    lhsT = ws[name][t][:, :, P * m:P * (m + 1)]
                    rhs = xt[t][:, :, csl]
                else:
                    lhsT = ws[name][t][:, P * m:P * (m + 1)]
                    rhs = xt[t][:, csl]
                mm(ps[:], lhsT, rhs, t == 0, t == NDT - 1)
            nc.vector.tensor_scalar_add(lst[m][:, csl], ps[:],
                                        biases[(bname, m)][:])

        ot_tiles = {}

        def wo_unit(i, e):
            ps = qkv_ps.tile([P, CHUNK], F32, tag="proj", name="proj")
            esl = slice(CHUNK * e, CHUNK * (e + 1))
            for m in range(NMT):
                nc.tensor.matmul(ps[:], ctxT[m][:, P * i:P * (i + 1)],
                                 wo[m][:, esl],
                                 start=(m == 0), stop=(m == NMT - 1))
            if i not in ot_tiles:
                ot_tiles[i] = out_p.tile([P, D], F16, tag="ot", name="ot")
            ot = ot_tiles[i]
            nc.vector.tensor_copy(ot[:, esl], ps[:])
            if e == 1:
                nc.sync.dma_start(o_d.ap()[P * i:P * (i + 1), :], ot[:])
                del ot_tiles[i]

        # ---- attention groups -------------------------------------------
        # group = (pair, hh, j0): two k tiles {j0, j0+1}, trimmed to the
        # valid q suffix W = CHUNK - 128*dd0 (dd0 = j0 - 4ci if diagonal),
        # packed compactly: ST cols [gi*W, gi*W+W).
        def st_exp_group(ci, pair, hh, j0, W):
            qoff = CHUNK - W
            psl = slice(HD * hh, HD * (hh + 1))
            qsl = slice(CHUNK * ci + qoff, CHUNK * (ci + 1))
            st = stp_ps.tile([P, 2 * CHUNK], F32, tag="stp", name="stp")
            for gi, j in enumerate((j0, j0 + 1)):
                nc.tensor.matmul(
                    st[:, W * gi:W * (gi + 1)],
                    kt[pair][psl, P * j:P * (j + 1)],
                    qt[pair][psl, qsl],
                    start=True, stop=True)
            p_t = ptile_p.tile([P, 2 * CHUNK], F16, tag="ptile",
                               name="ptile")
            nc.scalar.activation(
                p_t[:, 0:2 * W], st[:, 0:2 * W],
                mybir.ActivationFunctionType.Exp, scale=EXP_SCALE)
            if j0 >= 4 * ci and "mask" not in ablate:
                # gi=0 (dd=dd0): triangle in cols [0,128)
                nc.gpsimd.affine_select(
                    out=p_t[:, 0:P], in_=p_t[:, 0:P],
                    compare_op=mybir.AluOpType.is_ge,
                    fill=0.0, base=0, pattern=[[1, P]],
                    channel_multiplier=-1)
                # gi=1 (dd=dd0+1): zero cols [W,W+128), triangle next 128
                nc.gpsimd.affine_select(
                    out=p_t[:, W:W + 2 * P], in_=p_t[:, W:W + 2 * P],
                    compare_op=mybir.AluOpType.is_ge,
                    fill=0.0, base=-P, pattern=[[1, 2 * P]],
                    channel_multiplier=-1)
            return p_t

        def pv_group(ci, pair, hh, j0, W, p_t, pv, jmax):
            qoff = CHUNK - W
            for gi, j in enumerate((j0, j0 + 1)):
                h = 2 * pair + hh
                nc.tensor.matmul(
                    pv[:, qoff:CHUNK],
                    vaug[j][:, h, :],
                    p_t[:, W * gi:W * (gi + 1)],
                    start=(j == 0), stop=(j == jmax))

        def division_hh(ci, pair, hh, pv_t):
            # normalize one head's ctx^T as soon as its PV chain completes
            qsl = slice(CHUNK * ci, CHUNK * (ci + 1))
            den = den_p.tile([1, CHUNK], F32, tag="den", name="den")
            dinv = den_p.tile([1, CHUNK], F32, tag="dinv", name="dinv")
            recb = den_p.tile([HD, CHUNK], F32, tag="recb", name="recb")
            nc.vector.tensor_copy(den[0:1, :], pv_t[HD:HD + 1, :])
            nc.vector.reciprocal(dinv[:], den[:])
            nc.gpsimd.partition_broadcast(recb[0:HD, :], dinv[0:1, :])
            nc.vector.tensor_mul(ctxT[pair][HD * hh:HD * (hh + 1), qsl],
                                 pv_t[0:HD, :], recb[0:HD, :])

        # ---- main schedule ----------------------------------------------
        # Global software pipeline: PV(group) is emitted LOOKAHEAD groups
        # after its ST/exp, with one PE filler unit woven in per slot.
        from collections import deque

        LOOKAHEAD = _CACHE.get("lookahead", 3)
        # prologue: chunk 0 projections
        for j in range(4):
            v_proj(j)
        for name in ("wq", "wk"):
            for m in range(NMT):
                qk_proj(name, m, 0)

        fillers = deque()
        pend = deque()  # (ci, pair, hh, j0, W, p_t)
        pv_tiles = {}

        def pop_one():
            ci, pair, hh, j0, W, p_t = pend.popleft()
            jmax = 4 * ci + 3
            key = (pair, hh)
            if key not in pv_tiles:
                pv_tiles[key] = pv_ps.tile([HD + 1, CHUNK], F32,
                                           tag="pv", name="pv")
            pv_group(ci, pair, hh, j0, W, p_t, pv_tiles[key], jmax)
            if j0 + 1 == jmax:
                division_hh(ci, pair, hh, pv_tiles.pop(key))

        for ci in range(N_CH):
            jmax = 4 * ci + 3
            # chunk ci's STs consume qk/v projections queued as fillers in
            # chunk ci-1; any leftovers MUST be emitted before the first ST
            # (engines run their streams in order).
            while fillers:
                fillers.popleft()()
            if ci + 1 < N_CH:
                for j in range(4 * (ci + 1), 4 * (ci + 1) + 4):
                    fillers.append(lambda j=j: v_proj(j))
                for name in ("wq", "wk"):
                    for m in range(NMT):
                        fillers.append(
                            lambda name=name, m=m, cn=ci + 1:
                            qk_proj(name, m, cn))
            if ci > 0:
                for qi in range(4):
                    for e in range(2):
                        i = 4 * (ci - 1) + qi
                        fillers.append(lambda i=i, e=e: wo_unit(i, e))
            groups = [(pair, hh, j0)
                      for pair in range(NMT) for hh in range(2)
                      for j0 in range(0, jmax + 1, 2)]
            # fractional pacing: spread available fillers over this chunk's
            # pipeline slots
            slots = len(groups) - max(0, LOOKAHEAD - len(pend))
            rate = len(fillers) / max(1, slots)
            credit = 0.0
            for pair, hh, j0 in groups:
                dd0 = max(0, j0 - 4 * ci)
                W = CHUNK - P * dd0
                p_t = st_exp_group(ci, pair, hh, j0, W)
                pend.append((ci, pair, hh, j0, W, p_t))
                if len(pend) > LOOKAHEAD:
                    credit += rate
                    while credit >= 1.0 and fillers:
                        fillers.popleft()()
                        credit -= 1.0
                    pop_one()

        # drain the pipeline, then Wo for the last chunk (must come after
        # its divisions -- no weaving here)
        while pend:
            if fillers:
                fillers.popleft()()
            pop_one()
        for f in list(fillers):
            f()
        fillers.clear()
        for qi in range(4):
            for e in range(2):
                wo_unit(4 * (N_CH - 1) + qi, e)


def _emit_pv(prev, pv_tiles, pv_ps, pv_group, division, ci, jmax,
             last=False):
    pair, hh, j0, W, p_t = prev
    key = (pair, hh)
    if key not in pv_tiles:
        pv_tiles[key] = pv_ps.tile([HD + 1, CHUNK], F32,
                                   tag="pv", name="pv")
    pv_group(ci, pair, hh, j0, W, p_t, pv_tiles[key], jmax)
    if j0 + 1 == jmax and (hh == 1 or last):
        # both heads of this pair are done -> divide
        if (pair, 0) in pv_tiles and (pair, 1) in pv_tiles:
            division(ci, pair, [pv_tiles.pop((pair, 0)),
                                pv_tiles.pop((pair, 1))])


def _shard_inputs(x, Wq, bq, Wk, bk, Wv, bv, Wo, bo):
    mm_dt = _CACHE.get("mm_dt", "f16")
    fp8 = mm_dt == "fp8dr"
    import ml_dtypes
    ndt = ml_dtypes.float8_e4m3 if fp8 else np.float16
    x = np.asarray(x, np.float32)
    in_maps = []

    def pack_dr(a):  # [D, C] -> [64, NDT*2*C] paired rows
        Dd, C = a.shape
        a = a.reshape(NDT, P // 2, 2, C).transpose(1, 0, 2, 3)
        return np.ascontiguousarray(a.reshape(P // 2, NDT * 2 * C))

    for core in range(N_CORES):
        b, g = divmod(core, 4)
        ds = slice(DG * g, DG * (g + 1))
        xT = np.ascontiguousarray(x[b].T)
        wcat = np.concatenate(
            [np.asarray(W, np.float32)[:, ds] * WSCALE
             for W in (Wv, Wq, Wk)], axis=1)  # [D, 3*DG] (wv|wq|wk)
        if fp8:
            m = {"xd": pack_dr(xT).astype(ndt),
                 "wcat": pack_dr(wcat).astype(ndt)}
        else:
            m = {"xd": xT.astype(ndt),
                 "wcat": np.ascontiguousarray(wcat).astype(ndt)}
        m["wo"] = np.ascontiguousarray(
            np.asarray(Wo, np.float32)[ds, :]).astype(np.float16)
        bq32 = (np.asarray(bq, np.float32)[ds] * WSCALE).reshape(NMT, P).T
        bk32 = (np.asarray(bk, np.float32)[ds] * WSCALE).reshape(NMT, P).T
        m["bcat"] = np.ascontiguousarray(
            np.concatenate([bq32, bk32], axis=1))  # [P, 4]
        in_maps.append(m)
    return in_maps


def kernel(x, Wq, bq, Wk, bk, Wv, bv, Wo, bo):
    mm_dt = _CACHE.get("mm_dt", "f16")
    _CACHE["mm_dt"] = mm_dt
    if "nc" not in _CACHE:
        _CACHE["nc"] = build_kernel(mm_dt)
    nc = _CACHE["nc"]
    in_maps = _shard_inputs(x, Wq, bq, Wk, bk, Wv, bv, Wo, bo)
    res = run_bass_kernel_spmd(
        nc, in_maps, core_ids=list(range(N_CORES)), trace=False)
    out = np.zeros((B, S, D), np.float32)
    for core in range(N_CORES):
        out[core // 4] += res.results[core]["o"]
    # exact bias folding: +bo, + bv @ Wo (constant row vector)
    out += (np.asarray(bo, np.float32)
            + np.asarray(bv, np.float32) @ np.asarray(Wo, np.float32))
    return out


# revision 35
# speedup vs baseline: 1.4085x; 1.0418x over previous
"""Causal multi-head attention on 8 trn2 NeuronCores.

Problem: B=2, S=2048, D=1024, H=16 heads, HD=64. fp32 in/out.

Sharding: 8 cores = 2 (batch) x 4 (head groups of 4 heads).

v2 design (per core, batch b / head group g), default mode "f16":
  - Everything (x, Wq/Wk/Wv/Wo, Q^T/K^T/P/V/ctx^T) is fp16 on device:
    1 cyc/row PE rate at any moving width, DVE 2x on sbuf-only copies,
    half the HBM traffic of f32.  Weights are host-scaled by 32 and the
    factor compensated exactly (exp scale 2^-13; V_aug ones column 32.0
    so ctx = pv/den is exact).  An "fp8dr" DoubleRow mode exists but its
    ~2.4%/element quantization noise exceeds the 2e-2 budget -- unused.
  - Causal handling: per 512-wide q chunk, k tiles j<4ci are full; the 4
    diagonal j's are processed in pairs trimmed to the valid q suffix
    (width W=512-128*dd0, packed compactly so exp is one ACT call), with
    one gpsimd affine_select per diagonal j zeroing the remaining triangle.
  - Softmax denominator: V_aug ones column -> pv psum row 64; per head:
    DVE copy to a base-0 [1,512] row, reciprocal, gpsimd
    partition_broadcast to [64,512], DVE multiply writes ctx^T fp16
    (emitted per-hh so pv psum slots free early).
  - Output: Wo matmuls accumulate in PSUM, DVE-copy to fp16 sbuf, DMA out
    (hw gpsimd cannot read PSUM; host converts/sums in f32).
  - Emission schedule: global software pipeline across chunks; PV(group)
    is emitted LOOKAHEAD groups after its ST/exp with projection/Wo chains
    woven between at a fractional pacing rate, so the in-order Tensor
    stream never waits on ACT exp.  Loads are t-interleaved across both
    HWDGE queues (SP + Activation), one DMA per [128,*] tile, w
    concatenated (wv|wq|wk).  Tail Wo units rotate psum across the idle
    stp pool.
Host: sums the 4 head-group partials per batch and adds bo + bv @ Wo.
Measured: sim 147.3us; HW 208us (pre tail/prologue polish) vs 264us
baseline.
"""

import sys

if "/opt/trn_rl_repo" not in sys.path:
    sys.path.insert(0, "/opt/trn_rl_repo")

import numpy as np

import concourse.bacc as bacc
import concourse.bass as bass
import concourse.mybir as mybir
import concourse.tile as tile
from concourse.bass_utils import run_bass_kernel_spmd

B, S, D, H = 2, 2048, 1024, 16
HD = D // H  # 64
N_CORES = 8
HEADS_PER_CORE = H // 4  # 4
DG = HEADS_PER_CORE * HD  # 256 head dims per core
P = 128
CHUNK = 512  # q chunk width
N_KT = S // P  # 16 k tiles
N_CH = S // CHUNK  # 4 q chunks
NDT = D // P  # 8 contraction tiles over D
NMT = DG // P  # 2 m-tiles (head pairs)
F32 = mybir.dt.float32
F16 = mybir.dt.float16
FP8 = mybir.dt.float8e4
WSCALE = 32.0  # fp8 weight scale (exactly compensated downstream)
EXP_SCALE = 0.125 / (WSCALE * WSCALE)  # 2^-13

_CACHE = {}


def build_kernel(mm_dt="f16", unroll=1, ablate=()):
    fp8 = mm_dt == "fp8dr"
    nc = bacc.Bacc("TRN2", target_bir_lowering=False, debug=False)
    if fp8:
        x_d = nc.dram_tensor("xd", [P // 2, 2 * S * NDT], FP8,
                             kind="ExternalInput")
        w_d = nc.dram_tensor("wcat", [P // 2, 2 * 3 * DG * NDT], FP8,
                             kind="ExternalInput")
    else:
        x_d = nc.dram_tensor("xd", [D, S], F16, kind="ExternalInput")
        w_d = nc.dram_tensor("wcat", [D, 3 * DG], F16, kind="ExternalInput")
    wo_d = nc.dram_tensor("wo", [DG, D], F16, kind="ExternalInput")
    b_d = nc.dram_tensor("bcat", [P, 2 * NMT], F32, kind="ExternalInput")
    o_d = nc.dram_tensor("o", [S, D], F16, kind="ExternalOutput")

    with tile.TileContext(nc) as tc:
        def body(_iv=None):
            _body(tc, nc, fp8, x_d, w_d, wo_d, b_d, o_d, ablate)

        if unroll > 1:
            with tc.For_i(0, unroll, 1):
                body()
        else:
            body()

    nc.compile()
    return nc


def _body(tc, nc, fp8, x_d, w_d, wo_d, b_d, o_d, ablate=()):
    import contextlib
    ctx = contextlib.ExitStack()
    DR = mybir.MatmulPerfMode.DoubleRow if fp8 else None
    idt = FP8 if fp8 else F16
    with ctx:
        const = ctx.enter_context(tc.tile_pool(name="const", bufs=1))
        sbuf = ctx.enter_context(tc.tile_pool(name="sbuf", bufs=1))
        ptile_p = ctx.enter_context(tc.tile_pool(
            name="ptile", bufs=_CACHE.get("lookahead", 3) + 2))
        den_p = ctx.enter_context(tc.tile_pool(name="den", bufs=2))
        out_p = ctx.enter_context(tc.tile_pool(name="outp", bufs=3))
        pv_bufs, qkv_bufs = _CACHE.get("psum_cfg", (2, 2))
        stp_ps = ctx.enter_context(
            tc.tile_pool(name="stp_ps", bufs=2, space="PSUM"))
        pv_ps = ctx.enter_context(
            tc.tile_pool(name="pv_ps", bufs=pv_bufs, space="PSUM"))
        qkv_ps = ctx.enter_context(
            tc.tile_pool(name="qkv_ps", bufs=qkv_bufs, space="PSUM"))

        # ---- input tiles -------------------------------------------------
        # wq/wk/wv ship concatenated ([.., 3*DG] per contraction tile) so
        # each k-tile is ONE dma; x tiles are one dma each.  Loads alternate
        # between the two HWDGE queues (SP + Activation) t-interleaved so
        # the t=0 tiles land first; stores go on SP.
        WOFF = {"wv": 0, "wq": DG, "wk": 2 * DG}
        if fp8:
            xt = [const.tile([P // 2, 2, S], FP8, tag=f"xt{t}", name=f"xt{t}")
                  for t in range(NDT)]
            wct = [const.tile([P // 2, 2, 3 * DG], FP8, tag=f"wc{t}",
                              name=f"wc{t}") for t in range(NDT)]
            ws = {name: [wct[t][:, :, WOFF[name]:WOFF[name] + DG]
                         for t in range(NDT)]
                  for name in ("wq", "wk", "wv")}
        else:
            xt = [const.tile([P, S], F16, tag=f"xt{t}", name=f"xt{t}")
                  for t in range(NDT)]
            wct = [const.tile([P, 3 * DG], F16, tag=f"wc{t}",
                              name=f"wc{t}") for t in range(NDT)]
            ws = {name: [wct[t][:, WOFF[name]:WOFF[name] + DG]
                         for t in range(NDT)]
                  for name in ("wq", "wk", "wv")}
        wo = [const.tile([P, D], F16, tag=f"wo{m}", name=f"wo{m}")
              for m in range(NMT)]
        bcat = const.tile([P, 2 * NMT], F32, tag="bcat", name="bcat")
        biases = {(nm, m): bcat[:, i:i + 1]
                  for i, (nm, m) in enumerate(
                      (n, m) for n in ("bq", "bk") for m in range(NMT))}

        def x_src(t, csl):
            if fp8:
                return x_d.ap().rearrange("p (t two s) -> p t two s",
                                          t=NDT, two=2)[:, t, :, csl]
            return x_d.ap()[P * t:P * (t + 1), csl]

        def x_dst(t, csl):
            return xt[t][:, :, csl] if fp8 else xt[t][:, csl]

        for t in range(NDT):
            eng = nc.sync if t % 2 == 0 else nc.scalar
            if fp8:
                wsrc = w_d.ap().rearrange("p (t two g) -> p t two g",
                                          t=NDT, two=2)[:, t]
            else:
                wsrc = w_d.ap()[P * t:P * (t + 1), :]
            eng.dma_start(wct[t][:], wsrc)
            eng.dma_start(x_dst(t, slice(0, S)), x_src(t, slice(0, S)))
        nc.scalar.dma_start(bcat[:], b_d.ap()[:])
        for m in range(NMT):
            nc.scalar.dma_start(wo[m][:], wo_d.ap()[P * m:P * (m + 1), :])

        # ---- persistent sbuf tensors ------------------------------------
        qt = [sbuf.tile([P, S], F16, tag=f"qT{m}", name=f"qT{m}")
              for m in range(NMT)]
        kt = [sbuf.tile([P, S], F16, tag=f"kT{m}", name=f"kT{m}")
              for m in range(NMT)]
        ctxT = [sbuf.tile([P, S], F16, tag=f"ctxT{m}", name=f"ctxT{m}")
                for m in range(NMT)]
        vaug = [sbuf.tile([P, HEADS_PER_CORE, HD + 1], F16, tag=f"vaug{j}",
                          name=f"vaug{j}") for j in range(N_KT)]
        ones16 = const.tile([P, HEADS_PER_CORE, 1], F16, tag="ones16",
                            name="ones16")
        nc.vector.memset(ones16[:], WSCALE)

        # ---- projection / output chains (PE filler units) ---------------
        def mm(ps, lhsT, rhs, start, stop):
            nc.tensor.matmul(ps, lhsT, rhs, start=start, stop=stop,
                             perf_mode=DR)

        def v_proj(j):
            ps = qkv_ps.tile([P, CHUNK], F32, tag="proj", name="proj")
            for t in range(NDT):
                if fp8:
                    lhsT = xt[t][:, :, P * j:P * (j + 1)]
                    rhs = ws["wv"][t][:]
                else:
                    lhsT = xt[t][:, P * j:P * (j + 1)]
                    rhs = ws["wv"][t][:]
                mm(ps[:, 0:DG], lhsT, rhs, t == 0, t == NDT - 1)
            dst = vaug[j][:]
            srcp = ps[:, 0:DG].rearrange("p (h x) -> p h x",
                                         h=HEADS_PER_CORE)
            nc.vector.tensor_copy(dst[:, :, 0:HD], srcp)
            nc.vector.tensor_copy(dst[:, :, HD:HD + 1], ones16[:])

        def qk_proj(name, m, ci):
            lst = qt if name == "wq" else kt
            bname = "bq" if name == "wq" else "bk"
            csl = slice(CHUNK * ci, CHUNK * (ci + 1))
            ps = qkv_ps.tile([P, CHUNK], F32, tag="proj", name="proj")
            for t in range(NDT):
                if fp8:
                    lhsT = ws[name][t][:, :, P * m:P * (m + 1)]
                    rhs = xt[t][:, :, csl]
                else:
                    lhsT = ws[name][t][:, P * m:P * (m + 1)]
                    rhs = xt[t][:, csl]
                mm(ps[:], lhsT, rhs, t == 0, t == NDT - 1)
            nc.vector.tensor_scalar_add(lst[m][:, csl], ps[:],
                                        biases[(bname, m)][:])

        ot_tiles = {}

        def wo_unit(i, e, tail=False):
            if tail and e == 0:
                # after the last ST the stp banks are idle; rotating the
                # tail's Wo psum across both pools doubles tail overlap
                ps = stp_ps.tile([P, 2 * CHUNK], F32, tag="stp",
                                 name="stp")[:, 0:CHUNK]
            else:
                ps = qkv_ps.tile([P, CHUNK], F32, tag="proj", name="proj")
            esl = slice(CHUNK * e, CHUNK * (e + 1))
            for m in range(NMT):
                nc.tensor.matmul(ps[:], ctxT[m][:, P * i:P * (i + 1)],
                                 wo[m][:, esl],
                                 start=(m == 0), stop=(m == NMT - 1))
            if i not in ot_tiles:
                ot_tiles[i] = out_p.tile([P, D], F16, tag="ot", name="ot")
            ot = ot_tiles[i]
            nc.vector.tensor_copy(ot[:, esl], ps[:])
            if e == 1:
                nc.sync.dma_start(o_d.ap()[P * i:P * (i + 1), :], ot[:])
                del ot_tiles[i]

        # ---- attention groups -------------------------------------------
        # group = (pair, hh, j0): two k tiles {j0, j0+1}, trimmed to the
        # valid q suffix W = CHUNK - 128*dd0 (dd0 = j0 - 4ci if diagonal),
        # packed compactly: ST cols [gi*W, gi*W+W).
        def st_exp_group(ci, pair, hh, j0, W):
            qoff = CHUNK - W
            psl = slice(HD * hh, HD * (hh + 1))
            qsl = slice(CHUNK * ci + qoff, CHUNK * (ci + 1))
            st = stp_ps.tile([P, 2 * CHUNK], F32, tag="stp", name="stp")
            for gi, j in enumerate((j0, j0 + 1)):
                nc.tensor.matmul(
                    st[:, W * gi:W * (gi + 1)],
                    kt[pair][psl, P * j:P * (j + 1)],
                    qt[pair][psl, qsl],
                    start=True, stop=True)
            p_t = ptile_p.tile([P, 2 * CHUNK], F16, tag="ptile",
                               name="ptile")
            nc.scalar.activation(
                p_t[:, 0:2 * W], st[:, 0:2 * W],
                mybir.ActivationFunctionType.Exp, scale=EXP_SCALE)
            if j0 >= 4 * ci and "mask" not in ablate:
                # gi=0 (dd=dd0): triangle in cols [0,128)
                nc.gpsimd.affine_select(
                    out=p_t[:, 0:P], in_=p_t[:, 0:P],
                    compare_op=mybir.AluOpType.is_ge,
                    fill=0.0, base=0, pattern=[[1, P]],
                    channel_multiplier=-1)
                # gi=1 (dd=dd0+1): zero cols [W,W+128), triangle next 128
                nc.gpsimd.affine_select(
                    out=p_t[:, W:W + 2 * P], in_=p_t[:, W:W + 2 * P],
                    compare_op=mybir.AluOpType.is_ge,
                    fill=0.0, base=-P, pattern=[[1, 2 * P]],
                    channel_multiplier=-1)
            return p_t

        def pv_group(ci, pair, hh, j0, W, p_t, pv, jmax):
            qoff = CHUNK - W
            for gi, j in enumerate((j0, j0 + 1)):
                h = 2 * pair + hh
                nc.tensor.matmul(
                    pv[:, qoff:CHUNK],
                    vaug[j][:, h, :],
                    p_t[:, W * gi:W * (gi + 1)],
                    start=(j == 0), stop=(j == jmax))

        def division_hh(ci, pair, hh, pv_t):
            # normalize one head's ctx^T as soon as its PV chain completes
            qsl = slice(CHUNK * ci, CHUNK * (ci + 1))
            den = den_p.tile([1, CHUNK], F32, tag="den", name="den")
            dinv = den_p.tile([1, CHUNK], F32, tag="dinv", name="dinv")
            recb = den_p.tile([HD, CHUNK], F32, tag="recb", name="recb")
            nc.vector.tensor_copy(den[0:1, :], pv_t[HD:HD + 1, :])
            nc.vector.reciprocal(dinv[:], den[:])
            nc.gpsimd.partition_broadcast(recb[0:HD, :], dinv[0:1, :])
            nc.vector.tensor_mul(ctxT[pair][HD * hh:HD * (hh + 1), qsl],
                                 pv_t[0:HD, :], recb[0:HD, :])

        # ---- main schedule ----------------------------------------------
        # Global software pipeline: PV(group) is emitted LOOKAHEAD groups
        # after its ST/exp, with one PE filler unit woven in per slot.
        from collections import deque

        LOOKAHEAD = _CACHE.get("lookahead", 3)
        # prologue: chunk 0 projections, ordered by first use (pair-0 q/k
        # first so its ST groups can start while the rest projects)
        qk_proj("wq", 0, 0)
        qk_proj("wk", 0, 0)
        for j in range(4):
            v_proj(j)
        qk_proj("wq", 1, 0)
        qk_proj("wk", 1, 0)

        fillers = deque()
        pend = deque()  # (ci, pair, hh, j0, W, p_t)
        pv_tiles = {}

        def pop_one():
            ci, pair, hh, j0, W, p_t = pend.popleft()
            jmax = 4 * ci + 3
            key = (pair, hh)
            if key not in pv_tiles:
                pv_tiles[key] = pv_ps.tile([HD + 1, CHUNK], F32,
                                           tag="pv", name="pv")
            pv_group(ci, pair, hh, j0, W, p_t, pv_tiles[key], jmax)
            if j0 + 1 == jmax:
                division_hh(ci, pair, hh, pv_tiles.pop(key))

        for ci in range(N_CH):
            jmax = 4 * ci + 3
            # chunk ci's STs consume qk/v projections queued as fillers in
            # chunk ci-1; any leftovers MUST be emitted before the first ST
            # (engines run their streams in order).
            while fillers:
                fillers.popleft()()
            if ci + 1 < N_CH:
                for j in range(4 * (ci + 1), 4 * (ci + 1) + 4):
                    fillers.append(lambda j=j: v_proj(j))
                for name in ("wq", "wk"):
                    for m in range(NMT):
                        fillers.append(
                            lambda name=name, m=m, cn=ci + 1:
                            qk_proj(name, m, cn))
            if ci > 0:
                for qi in range(4):
                    for e in range(2):
                        i = 4 * (ci - 1) + qi
                        fillers.append(lambda i=i, e=e: wo_unit(i, e))
            groups = [(pair, hh, j0)
                      for pair in range(NMT) for hh in range(2)
                      for j0 in range(0, jmax + 1, 2)]
            # fractional pacing: spread available fillers over this chunk's
            # pipeline slots
            slots = len(groups) - max(0, LOOKAHEAD - len(pend))
            rate = len(fillers) / max(1, slots)
            credit = 0.0
            for pair, hh, j0 in groups:
                dd0 = max(0, j0 - 4 * ci)
                W = CHUNK - P * dd0
                p_t = st_exp_group(ci, pair, hh, j0, W)
                pend.append((ci, pair, hh, j0, W, p_t))
                if len(pend) > LOOKAHEAD:
                    credit += rate
                    while credit >= 1.0 and fillers:
                        fillers.popleft()()
                        credit -= 1.0
                    pop_one()

        # drain the pipeline, then Wo for the last chunk (must come after
        # its divisions -- no weaving here)
        while pend:
            if fillers:
                fillers.popleft()()
            pop_one()
        for f in list(fillers):
            f()
        fillers.clear()
        for qi in range(4):
            for e in range(2):
                wo_unit(4 * (N_CH - 1) + qi, e, tail=True)


def _emit_pv(prev, pv_tiles, pv_ps, pv_group, division, ci, jmax,
             last=False):
    pair, hh, j0, W, p_t = prev
    key = (pair, hh)
    if key not in pv_tiles:
        pv_tiles[key] = pv_ps.tile([HD + 1, CHUNK], F32,
                                   tag="pv", name="pv")
    pv_group(ci, pair, hh, j0, W, p_t, pv_tiles[key], jmax)
    if j0 + 1 == jmax and (hh == 1 or last):
        # both heads of this pair are done -> divide
        if (pair, 0) in pv_tiles and (pair, 1) in pv_tiles:
            division(ci, pair, [pv_tiles.pop((pair, 0)),
                                pv_tiles.pop((pair, 1))])


def _shard_inputs(x, Wq, bq, Wk, bk, Wv, bv, Wo, bo):
    mm_dt = _CACHE.get("mm_dt", "f16")
    fp8 = mm_dt == "fp8dr"
    import ml_dtypes
    ndt = ml_dtypes.float8_e4m3 if fp8 else np.float16
    x = np.asarray(x, np.float32)
    in_maps = []

    def pack_dr(a):  # [D, C] -> [64, NDT*2*C] paired rows
        Dd, C = a.shape
        a = a.reshape(NDT, P // 2, 2, C).transpose(1, 0, 2, 3)
        return np.ascontiguousarray(a.reshape(P // 2, NDT * 2 * C))

    for core in range(N_CORES):
        b, g = divmod(core, 4)
        ds = slice(DG * g, DG * (g + 1))
        xT = np.ascontiguousarray(x[b].T)
        wcat = np.concatenate(
            [np.asarray(W, np.float32)[:, ds] * WSCALE
             for W in (Wv, Wq, Wk)], axis=1)  # [D, 3*DG] (wv|wq|wk)
        if fp8:
            m = {"xd": pack_dr(xT).astype(ndt),
                 "wcat": pack_dr(wcat).astype(ndt)}
        else:
            m = {"xd": xT.astype(ndt),
                 "wcat": np.ascontiguousarray(wcat).astype(ndt)}
        m["wo"] = np.ascontiguousarray(
            np.asarray(Wo, np.float32)[ds, :]).astype(np.float16)
        bq32 = (np.asarray(bq, np.float32)[ds] * WSCALE).reshape(NMT, P).T
        bk32 = (np.asarray(bk, np.float32)[ds] * WSCALE).reshape(NMT, P).T
        m["bcat"] = np.ascontiguousarray(
            np.concatenate([bq32, bk32], axis=1))  # [P, 4]
        in_maps.append(m)
    return in_maps


def kernel(x, Wq, bq, Wk, bk, Wv, bv, Wo, bo):
    mm_dt = _CACHE.get("mm_dt", "f16")
    _CACHE["mm_dt"] = mm_dt
    if "nc" not in _CACHE:
        _CACHE["nc"] = build_kernel(mm_dt)
    nc = _CACHE["nc"]
    in_maps = _shard_inputs(x, Wq, bq, Wk, bk, Wv, bv, Wo, bo)
    res = run_bass_kernel_spmd(
        nc, in_maps, core_ids=list(range(N_CORES)), trace=False)
    out = np.zeros((B, S, D), np.float32)
    for core in range(N_CORES):
        out[core // 4] += res.results[core]["o"]
    # exact bias folding: +bo, + bv @ Wo (constant row vector)
    out += (np.asarray(bo, np.float32)
            + np.asarray(bv, np.float32) @ np.asarray(Wo, np.float32))
    return out


# revision 38
# speedup vs baseline: 1.5882x; 1.1276x over previous
"""Causal multi-head attention on 8 trn2 NeuronCores.

Problem: B=2, S=2048, D=1024, H=16 heads, HD=64. fp32 in/out.

Sharding: 8 cores = 2 (batch) x 4 (head groups of 4 heads).

v2 design (per core, batch b / head group g), default mode "f16":
  - Everything (x, Wq/Wk/Wv/Wo, Q^T/K^T/P/V/ctx^T) is fp16 on device:
    1 cyc/row PE rate at any moving width, DVE 2x on sbuf-only copies,
    half the HBM traffic of f32.  Weights are host-scaled by 32 and the
    factor compensated exactly (exp scale 2^-13; V_aug ones column 32.0
    so ctx = pv/den is exact).  An "fp8dr" DoubleRow mode exists but its
    ~2.4%/element quantization noise exceeds the 2e-2 budget -- unused.
  - Causal handling: per 512-wide q chunk, k tiles j<4ci are full; the 4
    diagonal j's are processed in pairs trimmed to the valid q suffix
    (width W=512-128*dd0, packed compactly so exp is one ACT call), with
    one gpsimd affine_select per diagonal j zeroing the remaining triangle.
  - Softmax denominator: V_aug ones column -> pv psum row 64; per head:
    DVE copy to a base-0 [1,512] row, reciprocal, gpsimd
    partition_broadcast to [64,512], DVE multiply writes ctx^T fp16
    (emitted per-hh so pv psum slots free early).
  - Output: Wo matmuls accumulate in PSUM, DVE-copy to fp16 sbuf, DMA out
    (hw gpsimd cannot read PSUM; host converts/sums in f32).
  - Emission schedule: global software pipeline across chunks; PV(group)
    is emitted LOOKAHEAD groups after its ST/exp with projection/Wo chains
    woven between at a fractional pacing rate, so the in-order Tensor
    stream never waits on ACT exp.  Loads are t-interleaved across both
    HWDGE queues (SP + Activation), one DMA per [128,*] tile, w
    concatenated (wv|wq|wk).  Tail Wo units rotate psum across the idle
    stp pool.
Host: sums the 4 head-group partials per batch and adds bo + bv @ Wo.
Measured: sim 147.3us; HW 208us (pre tail/prologue polish) vs 264us
baseline.
"""

import sys

if "/opt/trn_rl_repo" not in sys.path:
    sys.path.insert(0, "/opt/trn_rl_repo")

import numpy as np

import concourse.bacc as bacc
import concourse.bass as bass
import concourse.mybir as mybir
import concourse.tile as tile
from concourse.bass_utils import run_bass_kernel_spmd

B, S, D, H = 2, 2048, 1024, 16
HD = D // H  # 64
N_CORES = 8
HEADS_PER_CORE = H // 4  # 4
DG = HEADS_PER_CORE * HD  # 256 head dims per core
P = 128
CHUNK = 512  # q chunk width
N_KT = S // P  # 16 k tiles
N_CH = S // CHUNK  # 4 q chunks
NDT = D // P  # 8 contraction tiles over D
NMT = DG // P  # 2 m-tiles (head pairs)
F32 = mybir.dt.float32
F16 = mybir.dt.float16
FP8 = mybir.dt.float8e4
WSCALE = 32.0  # fp8 weight scale (exactly compensated downstream)
EXP_SCALE = 0.125 / (WSCALE * WSCALE)  # 2^-13

_CACHE = {}


def build_kernel(mm_dt="f16", unroll=1, ablate=()):
    fp8 = mm_dt == "fp8dr"
    nc = bacc.Bacc("TRN2", target_bir_lowering=False, debug=False)
    if fp8:
        x_d = nc.dram_tensor("xd", [P // 2, 2 * S * NDT], FP8,
                             kind="ExternalInput")
        w_d = nc.dram_tensor("wcat", [P // 2, 2 * 3 * DG * NDT], FP8,
                             kind="ExternalInput")
    else:
        x_d = nc.dram_tensor("xd", [D, S], F16, kind="ExternalInput")
        w_d = nc.dram_tensor("wcat", [D, 3 * DG], F16, kind="ExternalInput")
    wo_d = nc.dram_tensor("wo", [DG, D], F16, kind="ExternalInput")
    b_d = nc.dram_tensor("bcat", [P, 2 * NMT], F32, kind="ExternalInput")
    o_d = nc.dram_tensor("o", [S, D], F16, kind="ExternalOutput")

    with tile.TileContext(nc) as tc:
        def body(_iv=None):
            _body(tc, nc, fp8, x_d, w_d, wo_d, b_d, o_d, ablate)

        if unroll > 1:
            with tc.For_i(0, unroll, 1):
                body()
        else:
            body()

    nc.compile()
    return nc


def _body(tc, nc, fp8, x_d, w_d, wo_d, b_d, o_d, ablate=()):
    import contextlib
    ctx = contextlib.ExitStack()
    DR = mybir.MatmulPerfMode.DoubleRow if fp8 else None
    idt = FP8 if fp8 else F16
    with ctx:
        const = ctx.enter_context(tc.tile_pool(name="const", bufs=1))
        sbuf = ctx.enter_context(tc.tile_pool(name="sbuf", bufs=1))
        ptile_p = ctx.enter_context(tc.tile_pool(
            name="ptile", bufs=_CACHE.get("lookahead", 3) + 2))
        den_p = ctx.enter_context(tc.tile_pool(name="den", bufs=2))
        out_p = ctx.enter_context(tc.tile_pool(name="outp", bufs=3))
        pv_bufs, qkv_bufs = _CACHE.get("psum_cfg", (2, 2))
        stp_ps = ctx.enter_context(
            tc.tile_pool(name="stp_ps", bufs=2, space="PSUM"))
        pv_ps = ctx.enter_context(
            tc.tile_pool(name="pv_ps", bufs=pv_bufs, space="PSUM"))
        qkv_ps = ctx.enter_context(
            tc.tile_pool(name="qkv_ps", bufs=qkv_bufs, space="PSUM"))

        # ---- input tiles -------------------------------------------------
        # wq/wk/wv ship concatenated ([.., 3*DG] per contraction tile) so
        # each k-tile is ONE dma; x tiles are one dma each.  Loads alternate
        # between the two HWDGE queues (SP + Activation) t-interleaved so
        # the t=0 tiles land first; stores go on SP.
        WOFF = {"wv": 0, "wq": DG, "wk": 2 * DG}
        if fp8:
            xt = [const.tile([P // 2, 2, S], FP8, tag=f"xt{t}", name=f"xt{t}")
                  for t in range(NDT)]
            wct = [const.tile([P // 2, 2, 3 * DG], FP8, tag=f"wc{t}",
                              name=f"wc{t}") for t in range(NDT)]
            ws = {name: [wct[t][:, :, WOFF[name]:WOFF[name] + DG]
                         for t in range(NDT)]
                  for name in ("wq", "wk", "wv")}
        else:
            xt = [const.tile([P, S], F16, tag=f"xt{t}", name=f"xt{t}")
                  for t in range(NDT)]
            wct = [const.tile([P, 3 * DG], F16, tag=f"wc{t}",
                              name=f"wc{t}") for t in range(NDT)]
            ws = {name: [wct[t][:, WOFF[name]:WOFF[name] + DG]
                         for t in range(NDT)]
                  for name in ("wq", "wk", "wv")}
        wo = [const.tile([P, D], F16, tag=f"wo{m}", name=f"wo{m}")
              for m in range(NMT)]
        bcat = const.tile([P, 2 * NMT], F32, tag="bcat", name="bcat")
        biases = {(nm, m): bcat[:, i:i + 1]
                  for i, (nm, m) in enumerate(
                      (n, m) for n in ("bq", "bk") for m in range(NMT))}

        def x_src(t, csl):
            if fp8:
                return x_d.ap().rearrange("p (t two s) -> p t two s",
                                          t=NDT, two=2)[:, t, :, csl]
            return x_d.ap()[P * t:P * (t + 1), csl]

        def x_dst(t, csl):
            return xt[t][:, :, csl] if fp8 else xt[t][:, csl]

        for t in range(NDT):
            eng = nc.sync if t % 2 == 0 else nc.scalar
            eng2 = nc.scalar if t % 2 == 0 else nc.sync
            if fp8:
                wsrc = w_d.ap().rearrange("p (t two g) -> p t two g",
                                          t=NDT, two=2)[:, t]
            else:
                wsrc = w_d.ap()[P * t:P * (t + 1), :]
            eng.dma_start(wct[t][:], wsrc)
            eng2.dma_start(x_dst(t, slice(0, S)), x_src(t, slice(0, S)))
        nc.scalar.dma_start(bcat[:], b_d.ap()[:])
        for m in range(NMT):
            nc.scalar.dma_start(wo[m][:], wo_d.ap()[P * m:P * (m + 1), :])

        # ---- persistent sbuf tensors ------------------------------------
        qt = [sbuf.tile([P, S], F16, tag=f"qT{m}", name=f"qT{m}")
              for m in range(NMT)]
        kt = [sbuf.tile([P, S], F16, tag=f"kT{m}", name=f"kT{m}")
              for m in range(NMT)]
        ctxT = [sbuf.tile([P, S], F16, tag=f"ctxT{m}", name=f"ctxT{m}")
                for m in range(NMT)]
        vaug = [sbuf.tile([P, HEADS_PER_CORE, HD + 1], F16, tag=f"vaug{j}",
                          name=f"vaug{j}") for j in range(N_KT)]
        ones16 = const.tile([P, HEADS_PER_CORE, 1], F16, tag="ones16",
                            name="ones16")
        nc.vector.memset(ones16[:], WSCALE)

        # ---- projection / output chains (PE filler units) ---------------
        def mm(ps, lhsT, rhs, start, stop):
            nc.tensor.matmul(ps, lhsT, rhs, start=start, stop=stop,
                             perf_mode=DR)

        def v_proj(j):
            ps = qkv_ps.tile([P, CHUNK], F32, tag="proj", name="proj")
            for t in range(NDT):
                if fp8:
                    lhsT = xt[t][:, :, P * j:P * (j + 1)]
                    rhs = ws["wv"][t][:]
                else:
                    lhsT = xt[t][:, P * j:P * (j + 1)]
                    rhs = ws["wv"][t][:]
                mm(ps[:, 0:DG], lhsT, rhs, t == 0, t == NDT - 1)
            dst = vaug[j][:]
            srcp = ps[:, 0:DG].rearrange("p (h x) -> p h x",
                                         h=HEADS_PER_CORE)
            nc.vector.tensor_copy(dst[:, :, 0:HD], srcp)
            nc.vector.tensor_copy(dst[:, :, HD:HD + 1], ones16[:])

        def qk_proj(name, m, ci):
            lst = qt if name == "wq" else kt
            bname = "bq" if name == "wq" else "bk"
            csl = slice(CHUNK * ci, CHUNK * (ci + 1))
            ps = qkv_ps.tile([P, CHUNK], F32, tag="proj", name="proj")
            for t in range(NDT):
                if fp8:
                    lhsT = ws[name][t][:, :, P * m:P * (m + 1)]
                    rhs = xt[t][:, :, csl]
                else:
                    lhsT = ws[name][t][:, P * m:P * (m + 1)]
                    rhs = xt[t][:, csl]
                mm(ps[:], lhsT, rhs, t == 0, t == NDT - 1)
            nc.vector.tensor_scalar_add(lst[m][:, csl], ps[:],
                                        biases[(bname, m)][:])

        ot_tiles = {}

        def wo_unit(i, e, tail=False):
            if tail and e == 0:
                # after the last ST the stp banks are idle; rotating the
                # tail's Wo psum across both pools doubles tail overlap
                ps = stp_ps.tile([P, 2 * CHUNK], F32, tag="stp",
                                 name="stp")[:, 0:CHUNK]
            else:
                ps = qkv_ps.tile([P, CHUNK], F32, tag="proj", name="proj")
            esl = slice(CHUNK * e, CHUNK * (e + 1))
            for m in range(NMT):
                nc.tensor.matmul(ps[:], ctxT[m][:, P * i:P * (i + 1)],
                                 wo[m][:, esl],
                                 start=(m == 0), stop=(m == NMT - 1))
            if i not in ot_tiles:
                ot_tiles[i] = out_p.tile([P, D], F16, tag="ot", name="ot")
            ot = ot_tiles[i]
            nc.vector.tensor_copy(ot[:, esl], ps[:])
            if e == 1:
                nc.sync.dma_start(o_d.ap()[P * i:P * (i + 1), :], ot[:])
                del ot_tiles[i]

        # ---- attention groups -------------------------------------------
        # group = (pair, hh, j0): two k tiles {j0, j0+1}, trimmed to the
        # valid q suffix W = CHUNK - 128*dd0 (dd0 = j0 - 4ci if diagonal),
        # packed compactly: ST cols [gi*W, gi*W+W).
        def st_exp_group(ci, pair, hh, j0, W):
            qoff = CHUNK - W
            psl = slice(HD * hh, HD * (hh + 1))
            qsl = slice(CHUNK * ci + qoff, CHUNK * (ci + 1))
            st = stp_ps.tile([P, 2 * CHUNK], F32, tag="stp", name="stp")
            for gi, j in enumerate((j0, j0 + 1)):
                nc.tensor.matmul(
                    st[:, W * gi:W * (gi + 1)],
                    kt[pair][psl, P * j:P * (j + 1)],
                    qt[pair][psl, qsl],
                    start=True, stop=True)
            p_t = ptile_p.tile([P, 2 * CHUNK], F16, tag="ptile",
                               name="ptile")
            nc.scalar.activation(
                p_t[:, 0:2 * W], st[:, 0:2 * W],
                mybir.ActivationFunctionType.Exp, scale=EXP_SCALE)
            if j0 >= 4 * ci and "mask" not in ablate:
                # gi=0 (dd=dd0): triangle in cols [0,128)
                nc.gpsimd.affine_select(
                    out=p_t[:, 0:P], in_=p_t[:, 0:P],
                    compare_op=mybir.AluOpType.is_ge,
                    fill=0.0, base=0, pattern=[[1, P]],
                    channel_multiplier=-1)
                # gi=1 (dd=dd0+1): zero cols [W,W+128), triangle next 128
                nc.gpsimd.affine_select(
                    out=p_t[:, W:W + 2 * P], in_=p_t[:, W:W + 2 * P],
                    compare_op=mybir.AluOpType.is_ge,
                    fill=0.0, base=-P, pattern=[[1, 2 * P]],
                    channel_multiplier=-1)
            return p_t

        def pv_group(ci, pair, hh, j0, W, p_t, pv, jmax):
            qoff = CHUNK - W
            for gi, j in enumerate((j0, j0 + 1)):
                h = 2 * pair + hh
                nc.tensor.matmul(
                    pv[:, qoff:CHUNK],
                    vaug[j][:, h, :],
                    p_t[:, W * gi:W * (gi + 1)],
                    start=(j == 0), stop=(j == jmax))

        def division_hh(ci, pair, hh, pv_t):
            # normalize one head's ctx^T as soon as its PV chain completes
            qsl = slice(CHUNK * ci, CHUNK * (ci + 1))
            den = den_p.tile([1, CHUNK], F32, tag="den", name="den")
            dinv = den_p.tile([1, CHUNK], F32, tag="dinv", name="dinv")
            recb = den_p.tile([HD, CHUNK], F32, tag="recb", name="recb")
            nc.vector.tensor_copy(den[0:1, :], pv_t[HD:HD + 1, :])
            nc.vector.reciprocal(dinv[:], den[:])
            nc.gpsimd.partition_broadcast(recb[0:HD, :], dinv[0:1, :])
            nc.vector.tensor_mul(ctxT[pair][HD * hh:HD * (hh + 1), qsl],
                                 pv_t[0:HD, :], recb[0:HD, :])

        # ---- main schedule ----------------------------------------------
        # Global software pipeline: PV(group) is emitted LOOKAHEAD groups
        # after its ST/exp, with one PE filler unit woven in per slot.
        from collections import deque

        LOOKAHEAD = _CACHE.get("lookahead", 3)
        # prologue: chunk 0 projections, ordered by first use (pair-0 q/k
        # first so its ST groups can start while the rest projects)
        qk_proj("wq", 0, 0)
        qk_proj("wk", 0, 0)
        for j in range(4):
            v_proj(j)
        qk_proj("wq", 1, 0)
        qk_proj("wk", 1, 0)

        fillers = deque()
        pend = deque()  # (ci, pair, hh, j0, W, p_t)
        pv_tiles = {}

        def pop_one():
            ci, pair, hh, j0, W, p_t = pend.popleft()
            jmax = 4 * ci + 3
            key = (pair, hh)
            if key not in pv_tiles:
                pv_tiles[key] = pv_ps.tile([HD + 1, CHUNK], F32,
                                           tag="pv", name="pv")
            pv_group(ci, pair, hh, j0, W, p_t, pv_tiles[key], jmax)
            if j0 + 1 == jmax:
                division_hh(ci, pair, hh, pv_tiles.pop(key))

        for ci in range(N_CH):
            jmax = 4 * ci + 3
            # chunk ci's STs consume qk/v projections queued as fillers in
            # chunk ci-1; any leftovers MUST be emitted before the first ST
            # (engines run their streams in order).
            while fillers:
                fillers.popleft()()
            if ci + 1 < N_CH:
                for j in range(4 * (ci + 1), 4 * (ci + 1) + 4):
                    fillers.append(lambda j=j: v_proj(j))
                for name in ("wq", "wk"):
                    for m in range(NMT):
                        fillers.append(
                            lambda name=name, m=m, cn=ci + 1:
                            qk_proj(name, m, cn))
            if ci > 0:
                for qi in range(4):
                    for e in range(2):
                        i = 4 * (ci - 1) + qi
                        fillers.append(lambda i=i, e=e: wo_unit(i, e))
            groups = [(pair, hh, j0)
                      for pair in range(NMT) for hh in range(2)
                      for j0 in range(0, jmax + 1, 2)]
            # fractional pacing: spread available fillers over this chunk's
            # pipeline slots; in the last chunk hold LOOKAHEAD units back
            # to cover the final drain
            slots = len(groups) - max(0, LOOKAHEAD - len(pend))
            avail = len(fillers)
            if ci == N_CH - 1:
                avail = max(0, avail - LOOKAHEAD)
            rate = avail / max(1, slots)
            credit = 0.0
            for pair, hh, j0 in groups:
                dd0 = max(0, j0 - 4 * ci)
                W = CHUNK - P * dd0
                p_t = st_exp_group(ci, pair, hh, j0, W)
                pend.append((ci, pair, hh, j0, W, p_t))
                if len(pend) > LOOKAHEAD:
                    credit += rate
                    while credit >= 1.0 and fillers:
                        fillers.popleft()()
                        credit -= 1.0
                    pop_one()

        # drain the pipeline, then Wo for the last chunk (must come after
        # its divisions -- no weaving here)
        while pend:
            if fillers:
                fillers.popleft()()
            pop_one()
        for f in list(fillers):
            f()
        fillers.clear()
        for qi in range(4):
            for e in range(2):
                wo_unit(4 * (N_CH - 1) + qi, e, tail=True)


def _emit_pv(prev, pv_tiles, pv_ps, pv_group, division, ci, jmax,
             last=False):
    pair, hh, j0, W, p_t = prev
    key = (pair, hh)
    if key not in pv_tiles:
        pv_tiles[key] = pv_ps.tile([HD + 1, CHUNK], F32,
                                   tag="pv", name="pv")
    pv_group(ci, pair, hh, j0, W, p_t, pv_tiles[key], jmax)
    if j0 + 1 == jmax and (hh == 1 or last):
        # both heads of this pair are done -> divide
        if (pair, 0) in pv_tiles and (pair, 1) in pv_tiles:
            division(ci, pair, [pv_tiles.pop((pair, 0)),
                                pv_tiles.pop((pair, 1))])


def _shard_inputs(x, Wq, bq, Wk, bk, Wv, bv, Wo, bo):
    mm_dt = _CACHE.get("mm_dt", "f16")
    fp8 = mm_dt == "fp8dr"
    import ml_dtypes
    ndt = ml_dtypes.float8_e4m3 if fp8 else np.float16
    x = np.asarray(x, np.float32)
    in_maps = []

    def pack_dr(a):  # [D, C] -> [64, NDT*2*C] paired rows
        Dd, C = a.shape
        a = a.reshape(NDT, P // 2, 2, C).transpose(1, 0, 2, 3)
        return np.ascontiguousarray(a.reshape(P // 2, NDT * 2 * C))

    for core in range(N_CORES):
        b, g = divmod(core, 4)
        ds = slice(DG * g, DG * (g + 1))
        xT = np.ascontiguousarray(x[b].T)
        wcat = np.concatenate(
            [np.asarray(W, np.float32)[:, ds] * WSCALE
             for W in (Wv, Wq, Wk)], axis=1)  # [D, 3*DG] (wv|wq|wk)
        if fp8:
            m = {"xd": pack_dr(xT).astype(ndt),
                 "wcat": pack_dr(wcat).astype(ndt)}
        else:
            m = {"xd": xT.astype(ndt),
                 "wcat": np.ascontiguousarray(wcat).astype(ndt)}
        m["wo"] = np.ascontiguousarray(
            np.asarray(Wo, np.float32)[ds, :]).astype(np.float16)
        bq32 = (np.asarray(bq, np.float32)[ds] * WSCALE).reshape(NMT, P).T
        bk32 = (np.asarray(bk, np.float32)[ds] * WSCALE).reshape(NMT, P).T
        m["bcat"] = np.ascontiguousarray(
            np.concatenate([bq32, bk32], axis=1))  # [P, 4]
        in_maps.append(m)
    return in_maps


def kernel(x, Wq, bq, Wk, bk, Wv, bv, Wo, bo):
    mm_dt = _CACHE.get("mm_dt", "f16")
    _CACHE["mm_dt"] = mm_dt
    if "nc" not in _CACHE:
        _CACHE["nc"] = build_kernel(mm_dt)
    nc = _CACHE["nc"]
    in_maps = _shard_inputs(x, Wq, bq, Wk, bk, Wv, bv, Wo, bo)
    res = run_bass_kernel_spmd(
        nc, in_maps, core_ids=list(range(N_CORES)), trace=False)
    out = np.zeros((B, S, D), np.float32)
    for core in range(N_CORES):
        out[core // 4] += res.results[core]["o"]
    # exact bias folding: +bo, + bv @ Wo (constant row vector)
    out += (np.asarray(bo, np.float32)
            + np.asarray(bv, np.float32) @ np.asarray(Wo, np.float32))
    return out
